# revision 1
# baseline (speedup 1.0000x reference)
"""Data-parallel CRF NLL loss on 8 Trainium2 NeuronCores.

Strategy (per sharding hint): pure data parallel — shard logits/y_true
along batch (4096 -> 8 x 512), replicate the 15x15 transition matrix,
per-core partial sum of the clipped NLL, mean taken on host.

Hardcoded problem shape: logits (4096, 512, 15) f32, y_true (4096, 512) i32,
transitions (15, 15) f32. Output: scalar f32 (mean NLL).
"""

import numpy as np
import jax
import jax.numpy as jnp
from functools import partial

PAD_LABEL = 15
NUM_TAGS = 15
N_CORES = 8

try:
    jax.config.update("jax_compilation_cache_dir", "/tmp/jaxcache")
    jax.config.update("jax_persistent_cache_min_compile_time_secs", 0.0)
except Exception:
    pass

_compiled = {}


def _shard_loss_sum(logits, y_true, transitions):
    # logits: (b, T, C) f32; y_true: (b, T) i32; transitions: (C, C)
    mask = y_true != PAD_LABEL                       # (b, T)
    alpha0 = logits[:, 0, :]                         # (b, C)
    emits = jnp.swapaxes(logits[:, 1:, :], 0, 1)     # (T-1, b, C)
    masks = jnp.swapaxes(mask[:, 1:], 0, 1)          # (T-1, b)

    def step(alpha, inp):
        emit_t, mask_t = inp
        # scores[b,i,j] = alpha[b,i] + trans[i,j] + emit_t[b,j]
        scores = alpha[:, :, None] + transitions[None, :, :] + emit_t[:, None, :]
        m = jnp.max(scores, axis=1)                  # (b, C)
        new_alpha = m + jnp.log(jnp.sum(jnp.exp(scores - m[:, None, :]), axis=1))
        alpha = jnp.where(mask_t[:, None], new_alpha, alpha)
        return alpha, None

    alpha, _ = jax.lax.scan(step, alpha0, (emits, masks))
    mA = jnp.max(alpha, axis=1)
    logZ = mA + jnp.log(jnp.sum(jnp.exp(alpha - mA[:, None]), axis=1))  # (b,)

    mask_f = mask.astype(jnp.float32)
    safe_labels = jnp.where(mask, y_true, 0)
    em = jnp.take_along_axis(logits, safe_labels[:, :, None], axis=2)[:, :, 0]
    emission_score = jnp.sum(em * mask_f, axis=1)
    prev, curr = safe_labels[:, :-1], safe_labels[:, 1:]
    tmask_f = (mask[:, 1:] & mask[:, :-1]).astype(jnp.float32)
    trans_scores = transitions[prev, curr]
    transition_score = jnp.sum(trans_scores * tmask_f, axis=1)
    path_score = emission_score + transition_score

    nll = jnp.clip(logZ - path_score, 0.0, 1000000.0)
    return jnp.sum(nll)


def _get_fn():
    if "fn" not in _compiled:
        _compiled["fn"] = jax.pmap(
            _shard_loss_sum, axis_name="x", in_axes=(0, 0, None)
        )
    return _compiled["fn"]


def kernel(logits, y_true, transitions):
    logits = np.asarray(logits, dtype=np.float32)
    y_true = np.asarray(y_true, dtype=np.int32)
    transitions = np.asarray(transitions, dtype=np.float32)
    B, T, C = logits.shape
    b = B // N_CORES
    lg = logits.reshape(N_CORES, b, T, C)
    yt = y_true.reshape(N_CORES, b, T)
    sums = _get_fn()(lg, yt, transitions)            # (8,) per-core NLL sums
    out = np.float32(np.sum(np.asarray(sums, dtype=np.float64)) / B)
    return np.asarray(out, dtype=np.float32)

